# revision 14
# baseline (speedup 1.0000x reference)
"""AssumeNegativeLoss Trainium2 kernel.

Math (per batch row b over vocab V):
    bce(x,t) = max(x,0) - x*t + log1p(exp(-|x|))      (= softplus(-x) when t=1,
                                                         = softplus(x)  when t=0)
    pos_sum  = sum_{v: t=1} bce = sum_v softplus(-(x + 255*(1-t)))
               (the +255 pushes t=0 terms to softplus(-big) == 0)
    sampled negatives (M random indices per row, gathered sub-arrays):
    neg_sum  = [sum_{m: t_s=0} softplus(x_s)] * true_neg_cnt / max(neg_cnt_s, 1)
    loss_b   = (4*pos_sum + neg_sum) / V;   output = mean_b loss_b

Sharding: data-parallel over the batch — 8 cores x 128 rows, one row per SBUF
partition. The heavy compute (softplus over all B*V elements + all reductions)
runs on-device; the host only re-encodes inputs (bf16 logits, uint8 inverted
labels) and extracts the M sampled columns per row (pure indexing) because this
stack's per-element indirect DMA is unreliable (non-deterministic offset
consumption beyond ~128 descriptors/instruction).

Device pipeline per 5000-col chunk:
    DVE : z = x + inv255            (bf16)
    ACT : u = exp(-z)               (f32)
    ACT : ln(1+u), accum -> pos partial sums (exact softplus, no LUT shortcuts)
    POOL: copy inv, accum -> sum(inv255) partial (true_neg = sum/255)
Sampled phase is the same pattern on the (128, 1024) gathered tiles.
"""

import sys

for _p in ("/opt/trn_rl_repo", "/root/.axon_site/_ro/trn_rl_repo"):
    if _p not in sys.path:
        sys.path.insert(0, _p)

import numpy as np

B, V, M = 1024, 50000, 1024
NCORES = 8
R = B // NCORES  # 128 rows per core == SBUF partitions
C = 5000         # vocab chunk
NCH = V // C     # 10 chunks
POS_LAMBDA = 4.0

_CACHE = {}
LAST_RESULTS = None
LAST_IN_MAPS = None


def _build_program():
    import concourse.bacc as bacc
    import concourse.tile as tile
    from concourse import mybir

    f32 = mybir.dt.float32
    bf16 = mybir.dt.bfloat16
    u8 = mybir.dt.uint8
    Act = mybir.ActivationFunctionType
    Op = mybir.AluOpType

    nc = bacc.Bacc("TRN2", target_bir_lowering=False, debug=False)
    xb_d = nc.dram_tensor("xb", [R, V], bf16, kind="ExternalInput")
    inv_d = nc.dram_tensor("inv", [R, V], bf16, kind="ExternalInput")
    xs_d = nc.dram_tensor("xs", [R, M], bf16, kind="ExternalInput")
    invs_d = nc.dram_tensor("invs", [R, M], bf16, kind="ExternalInput")
    loss_d = nc.dram_tensor("loss", [R, 1], f32, kind="ExternalOutput")

    with tile.TileContext(nc) as tc:
        with tc.tile_pool(name="main", bufs=2) as pool, \
             tc.tile_pool(name="one", bufs=1) as pool1:
            pos_strip = pool1.tile([R, NCH], f32)
            cnt_strip = pool1.tile([R, NCH], f32)
            junk = pool1.tile([R, C], f32, tag="junk")

            for k in range(NCH):
                sl = slice(k * C, (k + 1) * C)
                xt = pool.tile([R, C], bf16, tag="xt")
                nc.sync.dma_start(out=xt[:], in_=xb_d[:, sl])
                invt = pool.tile([R, C], bf16, tag="invt")
                nc.sync.dma_start(out=invt[:], in_=inv_d[:, sl])
                z = pool.tile([R, C], bf16, tag="z")
                nc.vector.tensor_tensor(out=z[:], in0=xt[:], in1=invt[:], op=Op.add)
                u = pool.tile([R, C], f32, tag="u")
                nc.scalar.activation(u[:], z[:], Act.Exp, bias=0.0, scale=-1.0)
                nc.scalar.activation(junk[:], u[:], Act.Ln, bias=1.0, scale=1.0,
                                     accum_out=pos_strip[:, k:k + 1])
                nc.vector.tensor_reduce(out=cnt_strip[:, k:k + 1], in_=invt[:],
                                        axis=mybir.AxisListType.X, op=Op.add)

            # ---- sampled phase (tiny: R x M) ----
            xst = pool1.tile([R, M], bf16)
            nc.sync.dma_start(out=xst[:], in_=xs_d[:])
            invst = pool1.tile([R, M], bf16)
            nc.sync.dma_start(out=invst[:], in_=invs_d[:])
            # f32 out: bf16 zs would quantize the t=0 samples' logits (ulp(255)=1)
            zs = pool1.tile([R, M], f32)
            nc.vector.tensor_tensor(out=zs[:], in0=xst[:], in1=invst[:], op=Op.add)
            us = pool1.tile([R, M], f32)
            # exp(zs - 255): t=0 -> exp(x), t=1 -> exp(x-255) ~= 0
            # (bias must be an AP: only 0.0/1.0 are preregistered const APs)
            nbias = pool1.tile([R, 1], f32)
            nc.vector.memset(nbias[:], -255.0)
            nc.scalar.activation(us[:], zs[:], Act.Exp, bias=nbias[:], scale=1.0)
            sjunk = pool1.tile([R, M], f32)
            sns = pool1.tile([R, 1], f32)
            nc.scalar.activation(sjunk[:], us[:], Act.Ln, bias=1.0, scale=1.0,
                                 accum_out=sns[:])
            sinv_sum = pool1.tile([R, 1], f32)
            nc.vector.tensor_reduce(out=sinv_sum[:], in_=invst[:],
                                    axis=mybir.AxisListType.X, op=Op.add)

            # ---- final per-row math ----
            pos_sum = pool1.tile([R, 1], f32)
            nc.vector.tensor_reduce(out=pos_sum[:], in_=pos_strip[:],
                                    axis=mybir.AxisListType.X, op=Op.add)
            tneg = pool1.tile([R, 1], f32)
            nc.vector.tensor_reduce(out=tneg[:], in_=cnt_strip[:],
                                    axis=mybir.AxisListType.X, op=Op.add)
            # true_neg = sum(inv)/255 ; sampled_neg_cnt = max(sum(invs)/255, 1)
            snc = pool1.tile([R, 1], f32)
            nc.vector.tensor_scalar(out=snc[:], in0=sinv_sum[:],
                                    scalar1=1.0 / 255.0, scalar2=1.0,
                                    op0=Op.mult, op1=Op.max)
            rec = pool1.tile([R, 1], f32)
            nc.vector.reciprocal(rec[:], snc[:])
            # neg = sns * (tneg/255) * rec
            t1 = pool1.tile([R, 1], f32)
            nc.vector.tensor_scalar(out=t1[:], in0=tneg[:], scalar1=1.0 / 255.0,
                                    scalar2=None, op0=Op.mult)
            t2 = pool1.tile([R, 1], f32)
            nc.vector.tensor_tensor(out=t2[:], in0=sns[:], in1=t1[:], op=Op.mult)
            neg = pool1.tile([R, 1], f32)
            nc.vector.tensor_tensor(out=neg[:], in0=t2[:], in1=rec[:], op=Op.mult)
            # loss = (4*pos + neg)/V
            lsum = pool1.tile([R, 1], f32)
            nc.vector.scalar_tensor_tensor(out=lsum[:], in0=pos_sum[:],
                                           scalar=POS_LAMBDA, in1=neg[:],
                                           op0=Op.mult, op1=Op.add)
            lout = pool1.tile([R, 1], f32)
            nc.vector.tensor_scalar(out=lout[:], in0=lsum[:], scalar1=1.0 / V,
                                    scalar2=None, op0=Op.mult)
            nc.sync.dma_start(out=loss_d[:], in_=lout[:])

    nc.compile()
    return nc


def kernel(logits, targets, rand_indices):
    global LAST_RESULTS, LAST_IN_MAPS
    import ml_dtypes
    from concourse import bass_utils

    if "nc" not in _CACHE:
        _CACHE["nc"] = _build_program()
    nc = _CACHE["nc"]

    logits = np.asarray(logits, dtype=np.float32)
    targets = np.asarray(targets)
    idx = np.asarray(rand_indices).astype(np.int64)

    xb = logits.astype(ml_dtypes.bfloat16)
    # 255*(1-t) as bf16 (0.0 / 255.0, both exact in bf16)
    inv = np.where(np.asarray(targets) < 1, np.float32(255.0),
                   np.float32(0.0)).astype(ml_dtypes.bfloat16)
    xs_full = np.take_along_axis(logits, idx, axis=1).astype(ml_dtypes.bfloat16)
    invs_full = np.take_along_axis(inv, idx, axis=1)

    in_maps = []
    for c in range(NCORES):
        rs = slice(c * R, (c + 1) * R)
        in_maps.append({
            "xb": xb[rs],
            "inv": inv[rs],
            "xs": xs_full[rs],
            "invs": invs_full[rs],
        })

    LAST_IN_MAPS = in_maps
    res = bass_utils.run_bass_kernel_spmd(nc, in_maps, core_ids=list(range(NCORES)))
    LAST_RESULTS = res
    rows = np.concatenate([res.results[c]["loss"][:, 0] for c in range(NCORES)])
    return np.float32(rows.mean())


# revision 19
# speedup vs baseline: 31.9945x; 31.9945x over previous
"""AssumeNegativeLoss Trainium2 kernel.

Math (per batch row b over vocab V):
    bce(x,t) = max(x,0) - x*t + log1p(exp(-|x|))      (= softplus(-x) when t=1,
                                                         = softplus(x)  when t=0)
    pos_sum  = sum_{v: t=1} bce = sum_v softplus(-(x + 255*(1-t)))
               (the +255 pushes t=0 terms to softplus(-big) == 0)
    sampled negatives (M random indices per row, gathered sub-arrays):
    neg_sum  = [sum_{m: t_s=0} softplus(x_s)] * true_neg_cnt / max(neg_cnt_s, 1)
    loss_b   = (4*pos_sum + neg_sum) / V;   output = mean_b loss_b

Sharding: data-parallel over the batch — 8 cores x 128 rows, one row per SBUF
partition. The heavy compute (softplus over all B*V elements + all reductions)
runs on-device; the host only re-encodes inputs (bf16 logits, uint8 inverted
labels) and extracts the M sampled columns per row (pure indexing) because this
stack's per-element indirect DMA is unreliable (non-deterministic offset
consumption beyond ~128 descriptors/instruction).

Device pipeline per 5000-col chunk:
    DVE : z = x + inv255            (bf16)
    ACT : u = exp(-z)               (f32)
    ACT : ln(1+u), accum -> pos partial sums (exact softplus, no LUT shortcuts)
    POOL: copy inv, accum -> sum(inv255) partial (true_neg = sum/255)
Sampled phase is the same pattern on the (128, 1024) gathered tiles.
"""

import sys

for _p in ("/opt/trn_rl_repo", "/root/.axon_site/_ro/trn_rl_repo"):
    if _p not in sys.path:
        sys.path.insert(0, _p)

import numpy as np

B, V, M = 1024, 50000, 1024
NCORES = 8
R = B // NCORES  # 128 rows per core == SBUF partitions
C = 5000         # vocab chunk
NCH = V // C     # 10 chunks
POS_LAMBDA = 4.0

_CACHE = {}
LAST_RESULTS = None
LAST_IN_MAPS = None


def _build_program(reps=1):
    import concourse.bacc as bacc
    import concourse.tile as tile
    from concourse import mybir

    f32 = mybir.dt.float32
    bf16 = mybir.dt.bfloat16
    Act = mybir.ActivationFunctionType
    Op = mybir.AluOpType

    nc = bacc.Bacc("TRN2", target_bir_lowering=False, debug=False)
    xb_d = nc.dram_tensor("xb", [R, V], bf16, kind="ExternalInput")
    inv_d = nc.dram_tensor("inv", [R, V], bf16, kind="ExternalInput")
    xs_d = nc.dram_tensor("xs", [R, M], bf16, kind="ExternalInput")
    invs_d = nc.dram_tensor("invs", [R, M], bf16, kind="ExternalInput")
    loss_d = nc.dram_tensor("loss", [R, 1], f32, kind="ExternalOutput")

    with tile.TileContext(nc) as tc:
        with tc.tile_pool(name="main", bufs=2) as pool, \
             tc.tile_pool(name="one", bufs=1) as pool1:
          for _rep in range(reps):
            pos_strip = pool1.tile([R, NCH], f32)
            cnt_strip = pool1.tile([R, NCH], f32)
            junk = pool1.tile([R, C], f32, tag="junk")

            for k in range(NCH):
                sl = slice(k * C, (k + 1) * C)
                # deep prefetch on the load tiles keeps the DMA queues ahead
                # of compute; z/u stay at 2 to fit SBUF (~160KB/partition).
                xt = pool.tile([R, C], bf16, tag="xt", bufs=4)
                nc.sync.dma_start(out=xt[:], in_=xb_d[:, sl])
                invt = pool.tile([R, C], bf16, tag="invt", bufs=4)
                nc.sync.dma_start(out=invt[:], in_=inv_d[:, sl])
                z = pool.tile([R, C], bf16, tag="z")
                nc.vector.tensor_tensor(out=z[:], in0=xt[:], in1=invt[:], op=Op.add)
                u = pool.tile([R, C], f32, tag="u")
                nc.scalar.activation(u[:], z[:], Act.Exp, bias=0.0, scale=-1.0)
                nc.scalar.activation(junk[:], u[:], Act.Ln, bias=1.0, scale=1.0,
                                     accum_out=pos_strip[:, k:k + 1])
                nc.vector.tensor_reduce(out=cnt_strip[:, k:k + 1], in_=invt[:],
                                        axis=mybir.AxisListType.X, op=Op.add)

            # ---- sampled phase (tiny: R x M) ----
            xst = pool1.tile([R, M], bf16)
            nc.sync.dma_start(out=xst[:], in_=xs_d[:])
            invst = pool1.tile([R, M], bf16)
            nc.sync.dma_start(out=invst[:], in_=invs_d[:])
            # f32 out: bf16 zs would quantize the t=0 samples' logits (ulp(255)=1)
            zs = pool1.tile([R, M], f32)
            nc.vector.tensor_tensor(out=zs[:], in0=xst[:], in1=invst[:], op=Op.add)
            us = pool1.tile([R, M], f32)
            # exp(zs - 255): t=0 -> exp(x), t=1 -> exp(x-255) ~= 0
            # (bias must be an AP: only 0.0/1.0 are preregistered const APs)
            nbias = pool1.tile([R, 1], f32)
            nc.vector.memset(nbias[:], -255.0)
            nc.scalar.activation(us[:], zs[:], Act.Exp, bias=nbias[:], scale=1.0)
            sjunk = pool1.tile([R, M], f32)
            sns = pool1.tile([R, 1], f32)
            nc.scalar.activation(sjunk[:], us[:], Act.Ln, bias=1.0, scale=1.0,
                                 accum_out=sns[:])
            sinv_sum = pool1.tile([R, 1], f32)
            nc.vector.tensor_reduce(out=sinv_sum[:], in_=invst[:],
                                    axis=mybir.AxisListType.X, op=Op.add)

            # ---- final per-row math ----
            pos_sum = pool1.tile([R, 1], f32)
            nc.vector.tensor_reduce(out=pos_sum[:], in_=pos_strip[:],
                                    axis=mybir.AxisListType.X, op=Op.add)
            tneg = pool1.tile([R, 1], f32)
            nc.vector.tensor_reduce(out=tneg[:], in_=cnt_strip[:],
                                    axis=mybir.AxisListType.X, op=Op.add)
            # true_neg = sum(inv)/255 ; sampled_neg_cnt = max(sum(invs)/255, 1)
            snc = pool1.tile([R, 1], f32)
            nc.vector.tensor_scalar(out=snc[:], in0=sinv_sum[:],
                                    scalar1=1.0 / 255.0, scalar2=1.0,
                                    op0=Op.mult, op1=Op.max)
            rec = pool1.tile([R, 1], f32)
            nc.vector.reciprocal(rec[:], snc[:])
            # neg = sns * (tneg/255) * rec
            t1 = pool1.tile([R, 1], f32)
            nc.vector.tensor_scalar(out=t1[:], in0=tneg[:], scalar1=1.0 / 255.0,
                                    scalar2=None, op0=Op.mult)
            t2 = pool1.tile([R, 1], f32)
            nc.vector.tensor_tensor(out=t2[:], in0=sns[:], in1=t1[:], op=Op.mult)
            neg = pool1.tile([R, 1], f32)
            nc.vector.tensor_tensor(out=neg[:], in0=t2[:], in1=rec[:], op=Op.mult)
            # loss = (4*pos + neg)/V
            lsum = pool1.tile([R, 1], f32)
            nc.vector.scalar_tensor_tensor(out=lsum[:], in0=pos_sum[:],
                                           scalar=POS_LAMBDA, in1=neg[:],
                                           op0=Op.mult, op1=Op.add)
            lout = pool1.tile([R, 1], f32)
            nc.vector.tensor_scalar(out=lout[:], in0=lsum[:], scalar1=1.0 / V,
                                    scalar2=None, op0=Op.mult)
            nc.sync.dma_start(out=loss_d[:], in_=lout[:])

    nc.compile()
    return nc


def kernel(logits, targets, rand_indices):
    global LAST_RESULTS, LAST_IN_MAPS
    import ml_dtypes
    from concourse import bass_utils

    if "nc" not in _CACHE:
        _CACHE["nc"] = _build_program()
    nc = _CACHE["nc"]

    logits = np.asarray(logits, dtype=np.float32)
    targets = np.asarray(targets)
    idx = np.asarray(rand_indices).astype(np.int64)

    xb = logits.astype(ml_dtypes.bfloat16)
    # 255*(1-t) as bf16 (0.0 / 255.0, both exact in bf16)
    inv = np.where(np.asarray(targets) < 1, np.float32(255.0),
                   np.float32(0.0)).astype(ml_dtypes.bfloat16)
    xs_full = np.take_along_axis(logits, idx, axis=1).astype(ml_dtypes.bfloat16)
    invs_full = np.take_along_axis(inv, idx, axis=1)

    in_maps = []
    for c in range(NCORES):
        rs = slice(c * R, (c + 1) * R)
        in_maps.append({
            "xb": xb[rs],
            "inv": inv[rs],
            "xs": xs_full[rs],
            "invs": invs_full[rs],
        })

    LAST_IN_MAPS = in_maps
    res = bass_utils.run_bass_kernel_spmd(nc, in_maps, core_ids=list(range(NCORES)))
    LAST_RESULTS = res
    rows = np.concatenate([res.results[c]["loss"][:, 0] for c in range(NCORES)])
    return np.float32(rows.mean())
